# revision 10
# baseline (speedup 1.0000x reference)
"""Trainium2 Bass kernel for sliding-window multi-head attention (F5-TTS style).

Sharding: 8 cores = 2 batches x 4 head-groups. Each core computes 4 heads
(256 inner cols) end-to-end: QKV projections, RoPE (head 0 only, via per-core
cos/sin data so the SPMD program is uniform), banded attention, and its
row-slice of the output projection. Host sums the 4 partial outputs per batch.

v2: all matmul operands bf16 (tolerance is 2e-2; bf16 lands ~3e-3), RoPE
interleaved into phase A per token-chunk, per-t softmax tail batched across
the 4 heads through one PSUM tile + reciprocal_approx_fast + one broadcast,
band multiplies split between DVE and GpSimd, drains split scalar/vector.
"""
import os
import numpy as np

B, N, D = 2, 2048, 1024
H, HD = 16, 64
HPC = 4            # heads per core
SLICE = HPC * HD   # 256 inner cols per core
QB = 256           # query block
SW = 512           # key strip width
KB = D // 128      # 8 contraction blocks
NCH = N // 512     # 4 token chunks

_CACHE = {}
_last_results = None  # set by kernel() for test harness introspection


# ----------------------------------------------------------------------------
# device program
# ----------------------------------------------------------------------------
def _build_program(bv_nonzero: bool, bqk_nonzero: bool, debug: bool = False):
    import concourse.bacc as bacc
    import concourse.mybir as mybir
    import concourse.tile as tile
    from contextlib import ExitStack

    f32 = mybir.dt.float32
    bf16 = mybir.dt.bfloat16
    AF = mybir.ActivationFunctionType
    OP = mybir.AluOpType

    nc = bacc.Bacc("TRN2", target_bir_lowering=False, debug=False)

    xT_d = nc.dram_tensor("xT", [D, N], bf16, kind="ExternalInput").ap()
    wq_d = nc.dram_tensor("wq", [D, SLICE], bf16, kind="ExternalInput").ap()
    wk_d = nc.dram_tensor("wk", [D, SLICE], bf16, kind="ExternalInput").ap()
    wv_d = nc.dram_tensor("wv", [D, SLICE], bf16, kind="ExternalInput").ap()
    wo_d = nc.dram_tensor("wo", [SLICE, D], bf16, kind="ExternalInput").ap()
    bqk_d = nc.dram_tensor("bqk", [128, 4], f32, kind="ExternalInput").ap()
    cos_d = nc.dram_tensor("cosT", [64, N], bf16, kind="ExternalInput").ap()
    sin_d = nc.dram_tensor("sinT", [64, N], bf16, kind="ExternalInput").ap()
    band_d = nc.dram_tensor("band", [128, 3 * 1024], bf16, kind="ExternalInput").ap()
    bvr_d = nc.dram_tensor("bvrow", [1, 512], f32, kind="ExternalInput").ap()
    out_d = nc.dram_tensor("out", [D, N], bf16, kind="ExternalOutput").ap()

    def strip_of(t):
        return min(max(t * QB - 128, 0), N - SW)

    with tile.TileContext(nc) as tc:
        top = ExitStack()
        const = top.enter_context(tc.tile_pool(name="const", bufs=1))
        packs = top.enter_context(tc.tile_pool(name="packs", bufs=1))

        # persistent SBUF state
        q_pack = packs.tile([64, 4 * N], bf16, name="q_pack")    # head h at free h*N
        k_pack = packs.tile([64, 4 * N], bf16, name="k_pack")
        v_sb = packs.tile([128, 16 * 260], bf16, name="v_sb")    # per n-tile: 4 heads x (64 v + 1 ones)
        aoT = packs.tile([128, 2 * N], bf16, name="aoT")         # attn out [inner, n], icb block at free icb*N

        cos_t = const.tile([64, N], bf16, name="cos_t")
        sin_t = const.tile([64, N], bf16, name="sin_t")
        band_t = const.tile([128, 3 * 1024], bf16, name="band_t")
        bqk_t = const.tile([128, 4], f32, name="bqk_t")
        wo_t = const.tile([128, 2 * D], bf16, name="wo_t")

        wpool = top.enter_context(tc.tile_pool(name="wpool", bufs=1))
        wq_t = wpool.tile([128, KB * SLICE], bf16, name="wq_t")
        wk_t = wpool.tile([128, KB * SLICE], bf16, name="wk_t")
        wv_t = wpool.tile([128, KB * SLICE], bf16, name="wv_t")

        # spread the startup DMAs across queues; weights first (phase A needs
        # wq/wk before anything else), odds and ends later.
        nc.scalar.dma_start(wq_t[:].rearrange("p (b s) -> p b s", b=KB),
                            wq_d[:].rearrange("(b p) s -> p b s", p=128))
        nc.sync.dma_start(wk_t[:].rearrange("p (b s) -> p b s", b=KB),
                          wk_d[:].rearrange("(b p) s -> p b s", p=128))
        nc.scalar.dma_start(wv_t[:].rearrange("p (b s) -> p b s", b=KB),
                            wv_d[:].rearrange("(b p) s -> p b s", p=128))
        nc.gpsimd.dma_start(cos_t[:], cos_d[:])
        nc.gpsimd.dma_start(sin_t[:], sin_d[:])
        nc.gpsimd.dma_start(band_t[:], band_d[:])
        nc.gpsimd.dma_start(bqk_t[:], bqk_d[:])
        nc.gpsimd.dma_start(wo_t[:].rearrange("p (b s) -> p b s", b=2),
                            wo_d[:].rearrange("(b p) s -> p b s", p=128))

        # ones columns of v_sb
        ones_ap = v_sb[:, 0:16 * 260].rearrange(
            "p (t h e) -> p t h e", t=16, h=HPC)[:, :, :, 64:65]
        nc.vector.memset(ones_ap, 1.0)

        if bv_nonzero:
            bv_row = const.tile([1, 512], f32, name="bv_row")
            nc.sync.dma_start(bv_row[:], bvr_d[:])
            bv_bc = const.tile([128, 512], f32, name="bv_bc")
            nc.gpsimd.partition_broadcast(bv_bc[:], bv_row[0:1, :])

        # ------------------------------------------------ phase A: projections + rope
        pa = ExitStack()
        xt_pool = pa.enter_context(tc.tile_pool(name="xt", bufs=2))
        ps_qk = pa.enter_context(tc.tile_pool(name="ps_qk", bufs=3, space="PSUM"))
        ps_v = pa.enter_context(tc.tile_pool(name="ps_v", bufs=2, space="PSUM"))
        rope = pa.enter_context(tc.tile_pool(name="rope", bufs=2))

        for ch in range(NCH):
            csl = slice(ch * 512, (ch + 1) * 512)
            xt = xt_pool.tile([128, KB * 512], bf16, tag="xt")
            nc.sync.dma_start(xt[:].rearrange("p (b s) -> p b s", b=KB),
                              xT_d[:, csl].rearrange("(b p) s -> p b s", p=128))
            for cb in range(2):
                for wi, (wt, pack, bcol) in enumerate(
                        ((wq_t, q_pack, cb), (wk_t, k_pack, 2 + cb))):
                    pq = ps_qk.tile([128, 512], f32, tag="pq")
                    for kb in range(KB):
                        nc.tensor.matmul(
                            pq[:],
                            wt[:, kb * SLICE + cb * 128: kb * SLICE + (cb + 1) * 128],
                            xt[:, kb * 512:(kb + 1) * 512],
                            start=(kb == 0), stop=(kb == KB - 1))
                    # heads 2cb (psum rows 0:64) and 2cb+1 (rows 64:128);
                    # split the two drains across scalar and vector engines
                    for par in range(2):
                        h = 2 * cb + par
                        dst = pack[:, h * N + ch * 512: h * N + (ch + 1) * 512]
                        src = pq[64 * par:64 * (par + 1), :]
                        if bqk_nonzero:
                            nc.scalar.activation(
                                dst, src, AF.Identity,
                                bias=bqk_t[64 * par:64 * (par + 1), bcol:bcol + 1])
                        elif par == 0:
                            nc.scalar.copy(dst, src)
                        else:
                            nc.vector.tensor_copy(dst, src)
            for ti in range(4):
                pv = ps_v.tile([128, 256], f32, tag="pv")
                for kb in range(KB):
                    nc.tensor.matmul(
                        pv[:],
                        xt[:, kb * 512 + ti * 128: kb * 512 + (ti + 1) * 128],
                        wv_t[:, kb * SLICE:(kb + 1) * SLICE],
                        start=(kb == 0), stop=(kb == KB - 1))
                if bv_nonzero:
                    nc.vector.tensor_tensor(pv[:], pv[:], bv_bc[:, 0:256], OP.add)
                nt = ch * 4 + ti
                dst = v_sb[:, nt * 260:(nt + 1) * 260].rearrange(
                    "p (h e) -> p h e", h=HPC)[:, :, 0:64]
                src = pv[:].rearrange("p (h e) -> p h e", h=HPC)
                if ti % 2 == 0:
                    nc.scalar.copy(dst, src)
                else:
                    nc.vector.tensor_copy(dst, src)
            # rope for this chunk (head 0 of q_pack/k_pack lives at free 0:N)
            for pack in (q_pack, k_pack):
                psl = pack[:, csl]
                sw = rope.tile([64, 512], bf16, tag="sw")
                nc.sync.dma_start(sw[0:32, :], psl[32:64, :])
                nc.sync.dma_start(sw[32:64, :], psl[0:32, :])
                m = rope.tile([64, 512], bf16, tag="m")
                nc.vector.tensor_tensor(m[:], sw[:], sin_t[:, csl], OP.mult)
                t2 = rope.tile([64, 512], bf16, tag="t2")
                nc.vector.tensor_tensor(t2[:], psl, cos_t[:, csl], OP.mult)
                nc.vector.tensor_tensor(psl, t2[:], m[:], OP.add)
        pa.close()

        # ------------------------------------------------ phase C: banded attention
        pc = ExitStack()
        ps_s = pc.enter_context(tc.tile_pool(name="ps_s", bufs=2, space="PSUM"))
        ps_o = pc.enter_context(tc.tile_pool(name="ps_o", bufs=2, space="PSUM"))
        exp_pool = pc.enter_context(tc.tile_pool(name="expp", bufs=3))
        r_pool = pc.enter_context(tc.tile_pool(name="rp", bufs=2))
        if debug:
            dbg_pool = pc.enter_context(tc.tile_pool(name="dbg", bufs=1))
            sums_dbg = dbg_pool.tile([1, 8 * 1024], f32, name="sums_dbg")
            r_dbg = dbg_pool.tile([1, 8 * 1024], f32, name="r_dbg")
            ex_dbg = dbg_pool.tile([128, 2048], bf16, name="ex_dbg")

        for t in range(N // QB):
            strip = strip_of(t)
            drel = t * QB - strip
            bidx = {0: 0, 128: 1, 256: 2}[drel]
            po = ps_o.tile([65, 4 * QB], f32, tag="po")
            for h in range(HPC):
                ps = ps_s.tile([128, 1024], f32, tag="ps_s")
                for c in range(4):
                    nc.tensor.matmul(
                        ps[:, c * 256:(c + 1) * 256],
                        k_pack[:, h * N + strip + c * 128: h * N + strip + (c + 1) * 128],
                        q_pack[:, h * N + t * QB: h * N + (t + 1) * QB],
                        start=True, stop=True)
                ex0 = exp_pool.tile([128, 1024], bf16, tag="ex0")
                nc.scalar.activation(ex0[:], ps[:], AF.Exp, scale=0.125)
                ex = exp_pool.tile([128, 1024], bf16, tag="ex")
                eng = nc.vector if h % 2 == 0 else nc.gpsimd
                eng.tensor_tensor(ex[:], ex0[:],
                                  band_t[:, bidx * 1024:(bidx + 1) * 1024],
                                  OP.mult)
                if debug and t == 4 and h < 2:
                    nc.vector.tensor_copy(ex_dbg[:, h * 1024:(h + 1) * 1024], ex[:])
                for c in range(4):
                    ktile = (strip + c * 128) // 128
                    nc.tensor.matmul(
                        po[:, h * QB:(h + 1) * QB],
                        v_sb[:, ktile * 260 + h * 65: ktile * 260 + h * 65 + 65],
                        ex[:, c * 256:(c + 1) * 256],
                        start=(c == 0), stop=(c == 3))
            # batched softmax tail: denominators for all 4 heads sit on
            # po row 64; one approx-reciprocal + one broadcast covers them.
            # softmax denominators: PSUM row 64 -> SBUF, DMA-spread across 64
            # partitions so the iterative reciprocal runs on 16 elems/lane,
            # DMA back to a row, broadcast to 64 partitions.
            s_row = r_pool.tile([1, 4 * QB], f32, tag="s_row")
            if t % 2 == 0:
                nc.scalar.copy(s_row[:], po[64:65, :])
            else:
                nc.vector.tensor_copy(s_row[:], po[64:65, :])
            s32 = r_pool.tile([64, 16], f32, tag="s32")
            nc.sync.dma_start(s32[:], s_row[:])
            r32 = r_pool.tile([64, 16], f32, tag="r32")
            nc.vector.reciprocal(r32[:], s32[:])
            r_row = r_pool.tile([1, 4 * QB], f32, tag="r_row")
            nc.sync.dma_start(r_row[:], r32[:])
            rb = r_pool.tile([64, 4 * QB], f32, tag="rb")
            nc.gpsimd.partition_broadcast(rb[:], r_row[0:1, :])
            if debug:
                nc.vector.tensor_copy(sums_dbg[0:1, t * 1024:(t + 1) * 1024],
                                      po[64:65, :])
                nc.vector.tensor_copy(r_dbg[0:1, t * 1024:(t + 1) * 1024],
                                      r_row[:])
            for h in range(HPC):
                # head h -> inner block h//2, partition half h%2
                nc.vector.tensor_tensor(
                    aoT[64 * (h % 2): 64 * (h % 2) + 64,
                        (h // 2) * N + t * QB: (h // 2) * N + (t + 1) * QB],
                    po[0:64, h * QB:(h + 1) * QB], rb[:, h * QB:(h + 1) * QB],
                    OP.mult)
        if debug:
            # dump intermediates to out_d and skip phase D
            nc.sync.dma_start(out_d[0:64, :], q_pack[:, 0:N])
            nc.sync.dma_start(out_d[64:128, :], k_pack[:, 0:N])
            nc.sync.dma_start(out_d[128:256, :], v_sb[:, 0:2048])
            nc.sync.dma_start(out_d[256:320, :], q_pack[:, N:2 * N])
            nc.sync.dma_start(out_d[320:384, :], k_pack[:, N:2 * N])
            nc.sync.dma_start(out_d[384:512, :], aoT[:, 0:N])
            nc.sync.dma_start(out_d[512:640, :], aoT[:, N:2 * N])
            sums_bf = dbg_pool.tile([1, 8 * 1024], bf16, name="sums_bf")
            r_bf = dbg_pool.tile([1, 8 * 1024], bf16, name="r_bf")
            nc.vector.tensor_copy(sums_bf[:], sums_dbg[:])
            nc.vector.tensor_copy(r_bf[:], r_dbg[:])
            for rr in range(4):
                nc.sync.dma_start(out_d[640 + rr:641 + rr, :],
                                  sums_bf[0:1, rr * 2048:(rr + 1) * 2048])
                nc.sync.dma_start(out_d[644 + rr:645 + rr, :],
                                  r_bf[0:1, rr * 2048:(rr + 1) * 2048])
            nc.sync.dma_start(out_d[648:776, :], ex_dbg[:])
        pc.close()

        # ------------------------------------------------ phase D: output projection
        pd = ExitStack()
        ps_w = pd.enter_context(tc.tile_pool(name="ps_w", bufs=4, space="PSUM"))
        out_pool = pd.enter_context(tc.tile_pool(name="outp", bufs=3))
        for m in (range(0) if debug else range(8)):
            for half in range(2):
                ob = out_pool.tile([128, 1024], bf16, tag="ob")
                for sub in range(2):
                    ch = half * 2 + sub
                    pw = ps_w.tile([128, 512], f32, tag="pw")
                    for icb in range(2):
                        nc.tensor.matmul(
                            pw[:],
                            wo_t[:, icb * D + m * 128: icb * D + (m + 1) * 128],
                            aoT[:, icb * N + ch * 512: icb * N + (ch + 1) * 512],
                            start=(icb == 0), stop=(icb == 1))
                    if sub == 0:
                        nc.scalar.copy(ob[:, sub * 512:(sub + 1) * 512], pw[:])
                    else:
                        nc.vector.tensor_copy(ob[:, sub * 512:(sub + 1) * 512], pw[:])
                nc.gpsimd.dma_start(
                    out_d[m * 128:(m + 1) * 128, half * 1024:(half + 1) * 1024], ob[:])
        pd.close()
        top.close()

    nc.compile()
    return nc


# ----------------------------------------------------------------------------
# host side
# ----------------------------------------------------------------------------
def _host_prep(x, freqs, Wq, bq, Wk, bk, Wv, bv, Wo, half):
    """Build the 8 per-core input maps (bf16 device payloads)."""
    import ml_dtypes
    bf = ml_dtypes.bfloat16

    perm = np.concatenate([np.arange(0, 64, 2), np.arange(1, 64, 2)])
    cos_f = np.cos(freqs.astype(np.float64)).astype(np.float32)
    sin_f = np.sin(freqs.astype(np.float64)).astype(np.float32)
    cosT0 = np.ascontiguousarray(cos_f[:, perm].T)
    sinT0 = np.ascontiguousarray(sin_f[:, perm].T)
    sinT0[0:32] *= -1.0
    cos_id = np.ones((64, N), np.float32)
    sin_id = np.zeros((64, N), np.float32)

    # band patterns for the three strip offsets
    p = np.arange(128)
    q = np.arange(256)
    band = np.empty((128, 3 * 1024), np.float32)
    for bi, d in enumerate((0, 128, 256)):
        for c in range(4):
            k = c * 128 + p
            keep = np.abs(k[:, None] - d - q[None, :]) <= half
            band[:, bi * 1024 + c * 256: bi * 1024 + (c + 1) * 256] = \
                np.where(keep, 1.0, 0.0)

    bv_any = bool(np.any(bv))
    bqk_any = bool(np.any(bq) or np.any(bk))
    maps = []
    for core in range(8):
        b, g = core // 4, core % 4
        sl = slice(g * SLICE, (g + 1) * SLICE)
        wq_s = np.ascontiguousarray(Wq[:, sl])
        wk_s = np.ascontiguousarray(Wk[:, sl])
        bq_s = bq[sl].copy()
        bk_s = bk[sl].copy()
        if g == 0:
            wq_s = wq_s.copy(); wq_s[:, 0:64] = wq_s[:, 0:64][:, perm]
            wk_s = wk_s.copy(); wk_s[:, 0:64] = wk_s[:, 0:64][:, perm]
            bq_s[0:64] = bq_s[0:64][perm]
            bk_s[0:64] = bk_s[0:64][perm]
            cosT, sinT = cosT0, sinT0
        else:
            cosT, sinT = cos_id, sin_id
        # bias layout [128, 4]: cols (bq cb0, bq cb1, bk cb0, bk cb1)
        bqk = np.stack([bq_s[0:128], bq_s[128:256], bk_s[0:128], bk_s[128:256]],
                       axis=1).astype(np.float32)
        maps.append(dict(
            xT=np.ascontiguousarray(x[b].T).astype(bf),
            wq=wq_s.astype(bf), wk=wk_s.astype(bf),
            wv=np.ascontiguousarray(Wv[:, sl]).astype(bf),
            wo=np.ascontiguousarray(Wo[sl, :]).astype(bf),
            bqk=bqk, cosT=cosT.astype(bf), sinT=sinT.astype(bf),
            band=band.astype(bf),
            bvrow=np.concatenate([bv[sl], np.zeros(256, np.float32)])[None, :]
            .astype(np.float32),
        ))
    return maps, bv_any, bqk_any


def _numpy_fallback(x, mask, freqs, Wq, bq, Wk, bk, Wv, bv, Wo, bo, window_size):
    """Reference math in numpy (handles arbitrary mask / window)."""
    b, n, _ = x.shape
    h, hd = H, HD

    def rope(t):
        rot = freqs.shape[-1]
        tr = t[..., :rot].reshape(b, n, -1, 2)
        t1, t2 = tr[..., 0], tr[..., 1]
        rh = np.stack((-t2, t1), -1).reshape(b, n, rot)
        return np.concatenate(
            [t[..., :rot] * np.cos(freqs) + rh * np.sin(freqs), t[..., rot:]], -1)

    q = rope(x @ Wq + bq).reshape(b, n, h, hd).transpose(0, 2, 1, 3)
    k = rope(x @ Wk + bk).reshape(b, n, h, hd).transpose(0, 2, 1, 3)
    v = (x @ Wv + bv).reshape(b, n, h, hd).transpose(0, 2, 1, 3)
    i = np.arange(n)[:, None]
    j = np.arange(n)[None, :]
    half = int(window_size) // 2
    wm = (j >= i - half) & (j <= i + half)
    fm = wm[None, None] & mask[:, None, None, :]
    s = np.einsum("bhqd,bhkd->bhqk", q, k) / np.sqrt(np.float32(hd))
    s = np.where(fm, s, np.finfo(np.float32).min)
    s = s - s.max(-1, keepdims=True)
    e = np.exp(s)
    a = e / e.sum(-1, keepdims=True)
    out = np.einsum("bhqk,bhkd->bhqd", a, v).transpose(0, 2, 1, 3).reshape(b, n, h * hd)
    out = out @ Wo + bo
    return np.where(mask[..., None], out, 0.0).astype(np.float32)


def _ensure_ntff_hook():
    """The agent image's antenv lacks axon_hooks; synthesize it so
    run_bass_kernel_spmd(trace=True) can capture NTFF profiles."""
    import sys
    import types
    try:
        from antenv.axon_hooks import get_axon_ntff_profile_hook  # noqa: F401
        return
    except ImportError:
        pass
    try:
        import antenv
        from trn_agent_boot.trn_boot import _ntff_profile_via_ctypes
        hook = _ntff_profile_via_ctypes("/opt/axon/libaxon_pjrt.so")
        mod = types.ModuleType("antenv.axon_hooks")
        mod.get_axon_ntff_profile_hook = lambda: hook
        mod.set_axon_ntff_profile_hook = lambda h: None
        sys.modules["antenv.axon_hooks"] = mod
        antenv.axon_hooks = mod
    except Exception:
        pass


def kernel(x, mask, freqs, Wq, bq, Wk, bk, Wv, bv, Wo, bo, window_size):
    global _last_results
    x = np.asarray(x, np.float32)
    mask_np = np.asarray(mask)
    freqs = np.asarray(freqs, np.float32)
    Wq = np.asarray(Wq, np.float32); Wk = np.asarray(Wk, np.float32)
    Wv = np.asarray(Wv, np.float32); Wo = np.asarray(Wo, np.float32)
    bq = np.asarray(bq, np.float32); bk = np.asarray(bk, np.float32)
    bv = np.asarray(bv, np.float32); bo = np.asarray(bo, np.float32)
    ws = int(window_size)

    if (x.shape != (B, N, D) or freqs.shape != (N, HD) or ws > 256 or ws % 2
            or not mask_np.all()):
        return _numpy_fallback(x, mask_np, freqs, Wq, bq, Wk, bk, Wv, bv, Wo, bo, ws)

    from concourse.bass_utils import run_bass_kernel_spmd

    maps, bv_any, bqk_any = _host_prep(x, freqs, Wq, bq, Wk, bk, Wv, bv, Wo, ws // 2)
    dbg = bool(int(os.environ.get("KERNEL_DEBUG", "0")))
    key = ("v2", bv_any, bqk_any, dbg)
    if key not in _CACHE:
        _CACHE[key] = _build_program(bv_any, bqk_any, debug=dbg)
    nc = _CACHE[key]

    trace = bool(int(os.environ.get("KERNEL_TRACE", "0")))
    if trace:
        _ensure_ntff_hook()
    res = run_bass_kernel_spmd(nc, maps, core_ids=list(range(8)), trace=trace)
    _last_results = res

    out = np.empty((B, N, D), np.float32)
    for b in range(B):
        acc = res.results[4 * b]["out"].astype(np.float32)
        for g in range(1, 4):
            acc = acc + res.results[4 * b + g]["out"].astype(np.float32)
        out[b] = acc.T + bo[None, :]
    out *= mask_np[..., None].astype(np.float32)
    return out


# revision 16
# speedup vs baseline: 1.1236x; 1.1236x over previous
"""Trainium2 Bass kernel for sliding-window multi-head attention (F5-TTS style).

Sharding: 8 cores = 2 batches x 4 head-groups. Each core computes 4 heads
(256 inner cols) end-to-end: QKV projections, RoPE (head 0 only, via per-core
cos/sin data so the SPMD program is uniform), banded attention, and its
row-slice of the output projection. Host sums the 4 partial outputs per batch.

v2: all matmul operands bf16 (tolerance is 2e-2; bf16 lands ~3e-3), RoPE
interleaved into phase A per token-chunk, per-t softmax tail batched across
the 4 heads through one PSUM tile + reciprocal_approx_fast + one broadcast,
band multiplies split between DVE and GpSimd, drains split scalar/vector.
"""
import os
import numpy as np

B, N, D = 2, 2048, 1024
H, HD = 16, 64
HPC = 4            # heads per core
SLICE = HPC * HD   # 256 inner cols per core
QB = 256           # query block
SW = 512           # key strip width
KB = D // 128      # 8 contraction blocks
NCH = N // 512     # 4 token chunks

_CACHE = {}
_last_results = None  # set by kernel() for test harness introspection


# ----------------------------------------------------------------------------
# device program
# ----------------------------------------------------------------------------
def _build_program(bv_nonzero: bool, bqk_nonzero: bool, debug: bool = False):
    import concourse.bacc as bacc
    import concourse.mybir as mybir
    import concourse.tile as tile
    from contextlib import ExitStack

    f32 = mybir.dt.float32
    bf16 = mybir.dt.bfloat16
    AF = mybir.ActivationFunctionType
    OP = mybir.AluOpType

    nc = bacc.Bacc("TRN2", target_bir_lowering=False, debug=False)

    # all pre-swizzled host-side so every DMA is contiguous per partition
    xT_d = nc.dram_tensor("xT", [128, NCH * KB * 512], bf16, kind="ExternalInput").ap()
    wq_d = nc.dram_tensor("wq", [128, KB * SLICE], bf16, kind="ExternalInput").ap()
    wk_d = nc.dram_tensor("wk", [128, KB * SLICE], bf16, kind="ExternalInput").ap()
    wv_d = nc.dram_tensor("wv", [128, KB * SLICE], bf16, kind="ExternalInput").ap()
    wo_d = nc.dram_tensor("wo", [128, 2 * D], bf16, kind="ExternalInput").ap()
    bqk_d = nc.dram_tensor("bqk", [128, 4], f32, kind="ExternalInput").ap()
    cos_d = nc.dram_tensor("cosT", [64, N], bf16, kind="ExternalInput").ap()
    sin_d = nc.dram_tensor("sinT", [64, N], bf16, kind="ExternalInput").ap()
    band_d = nc.dram_tensor("band", [128, 3 * 1024], bf16, kind="ExternalInput").ap()
    bvr_d = nc.dram_tensor("bvrow", [1, 512], f32, kind="ExternalInput").ap()
    out_d = nc.dram_tensor("out", [D, N], bf16, kind="ExternalOutput").ap()

    def strip_of(t):
        return min(max(t * QB - 128, 0), N - SW)

    with tile.TileContext(nc) as tc:
        top = ExitStack()
        const = top.enter_context(tc.tile_pool(name="const", bufs=1))
        packs = top.enter_context(tc.tile_pool(name="packs", bufs=1))

        # persistent SBUF state
        q_pack = packs.tile([64, 4 * N], bf16, name="q_pack")    # head h at free h*N
        k_pack = packs.tile([64, 4 * N], bf16, name="k_pack")
        v_sb = packs.tile([128, 16 * 260], bf16, name="v_sb")    # per n-tile: 4 heads x (64 v + 1 ones)
        aoT = packs.tile([128, 2 * N], bf16, name="aoT")         # attn out [inner, n], icb block at free icb*N

        cos_t = const.tile([64, N], bf16, name="cos_t")
        sin_t = const.tile([64, N], bf16, name="sin_t")
        band_t = const.tile([128, 3 * 1024], bf16, name="band_t")
        bqk_t = const.tile([128, 4], f32, name="bqk_t")
        wo_t = const.tile([128, 2 * D], bf16, name="wo_t")

        wpool = top.enter_context(tc.tile_pool(name="wpool", bufs=1))
        wq_t = wpool.tile([128, KB * SLICE], bf16, name="wq_t")
        wk_t = wpool.tile([128, KB * SLICE], bf16, name="wk_t")
        wv_t = wpool.tile([128, KB * SLICE], bf16, name="wv_t")

        # spread the startup DMAs across queues; weights first (phase A needs
        # wq/wk before anything else), odds and ends later.
        nc.scalar.dma_start(wq_t[:], wq_d[:])
        nc.sync.dma_start(wk_t[:], wk_d[:])
        nc.scalar.dma_start(wv_t[:], wv_d[:])
        nc.gpsimd.dma_start(cos_t[:], cos_d[:])
        nc.gpsimd.dma_start(sin_t[:], sin_d[:])
        nc.gpsimd.dma_start(band_t[:], band_d[:])
        nc.gpsimd.dma_start(bqk_t[:], bqk_d[:])
        nc.gpsimd.dma_start(wo_t[:], wo_d[:])

        # ones columns of v_sb
        ones_ap = v_sb[:, 0:16 * 260].rearrange(
            "p (t h e) -> p t h e", t=16, h=HPC)[:, :, :, 64:65]
        nc.vector.memset(ones_ap, 1.0)

        if bv_nonzero:
            bv_row = const.tile([1, 512], f32, name="bv_row")
            nc.sync.dma_start(bv_row[:], bvr_d[:])
            bv_bc = const.tile([128, 512], f32, name="bv_bc")
            nc.gpsimd.partition_broadcast(bv_bc[:], bv_row[0:1, :])

        # ------------------------------------------------ phase A: projections + rope
        pa = ExitStack()
        pa.enter_context(nc.named_scope("phA"))
        xt_pool = pa.enter_context(tc.tile_pool(name="xt", bufs=2))
        ps_qk = pa.enter_context(tc.tile_pool(name="ps_qk", bufs=3, space="PSUM"))
        ps_v = pa.enter_context(tc.tile_pool(name="ps_v", bufs=2, space="PSUM"))
        rope = pa.enter_context(tc.tile_pool(name="rope", bufs=2))

        for ch in range(NCH):
            csl = slice(ch * 512, (ch + 1) * 512)
            xt = xt_pool.tile([128, KB * 512], bf16, tag="xt")
            nc.sync.dma_start(xt[:], xT_d[:, ch * KB * 512:(ch + 1) * KB * 512])
            for cb in range(2):
                for wi, (wt, pack, bcol) in enumerate(
                        ((wq_t, q_pack, cb), (wk_t, k_pack, 2 + cb))):
                    pq = ps_qk.tile([128, 512], f32, tag="pq")
                    for kb in range(KB):
                        nc.tensor.matmul(
                            pq[:],
                            wt[:, kb * SLICE + cb * 128: kb * SLICE + (cb + 1) * 128],
                            xt[:, kb * 512:(kb + 1) * 512],
                            start=(kb == 0), stop=(kb == KB - 1))
                    # heads 2cb (psum rows 0:64) and 2cb+1 (rows 64:128);
                    # split the two drains across scalar and vector engines
                    for par in range(2):
                        h = 2 * cb + par
                        dst = pack[:, h * N + ch * 512: h * N + (ch + 1) * 512]
                        src = pq[64 * par:64 * (par + 1), :]
                        if bqk_nonzero:
                            nc.scalar.activation(
                                dst, src, AF.Identity,
                                bias=bqk_t[64 * par:64 * (par + 1), bcol:bcol + 1])
                        elif par == 0:
                            nc.scalar.copy(dst, src)
                        else:
                            nc.vector.tensor_copy(dst, src)
            for ti in range(4):
                pv = ps_v.tile([128, 256], f32, tag="pv")
                for kb in range(KB):
                    nc.tensor.matmul(
                        pv[:],
                        xt[:, kb * 512 + ti * 128: kb * 512 + (ti + 1) * 128],
                        wv_t[:, kb * SLICE:(kb + 1) * SLICE],
                        start=(kb == 0), stop=(kb == KB - 1))
                if bv_nonzero:
                    nc.vector.tensor_tensor(pv[:], pv[:], bv_bc[:, 0:256], OP.add)
                nt = ch * 4 + ti
                dst = v_sb[:, nt * 260:(nt + 1) * 260].rearrange(
                    "p (h e) -> p h e", h=HPC)[:, :, 0:64]
                src = pv[:].rearrange("p (h e) -> p h e", h=HPC)
                if ti % 2 == 0:
                    nc.scalar.copy(dst, src)
                else:
                    nc.vector.tensor_copy(dst, src)
            # rope for this chunk (head 0 of q_pack/k_pack lives at free 0:N)
            for pack in (q_pack, k_pack):
                psl = pack[:, csl]
                sw = rope.tile([64, 512], bf16, tag="sw")
                nc.sync.dma_start(sw[0:32, :], psl[32:64, :])
                nc.sync.dma_start(sw[32:64, :], psl[0:32, :])
                m = rope.tile([64, 512], bf16, tag="m")
                nc.vector.tensor_tensor(m[:], sw[:], sin_t[:, csl], OP.mult)
                t2 = rope.tile([64, 512], bf16, tag="t2")
                nc.vector.tensor_tensor(t2[:], psl, cos_t[:, csl], OP.mult)
                nc.vector.tensor_tensor(psl, t2[:], m[:], OP.add)
        pa.close()

        # ------------------------------------------------ phase C: banded attention
        pc = ExitStack()
        pc.enter_context(nc.named_scope("phC"))
        ps_s = pc.enter_context(tc.tile_pool(name="ps_s", bufs=2, space="PSUM"))
        ps_o = pc.enter_context(tc.tile_pool(name="ps_o", bufs=2, space="PSUM"))
        exp_pool = pc.enter_context(tc.tile_pool(name="expp", bufs=3))
        r_pool = pc.enter_context(tc.tile_pool(name="rp", bufs=2))
        if debug:
            dbg_pool = pc.enter_context(tc.tile_pool(name="dbg", bufs=1))
            sums_dbg = dbg_pool.tile([1, 8 * 1024], f32, name="sums_dbg")
            r_dbg = dbg_pool.tile([1, 8 * 1024], f32, name="r_dbg")
            ex_dbg = dbg_pool.tile([128, 2048], bf16, name="ex_dbg")

        po_pool = pc.enter_context(tc.tile_pool(name="pos", bufs=3))

        def emit_compute(t):
            """scores -> exp -> band -> attn@v for all 4 heads of block t,
            then drain po (free dims reordered to (h%2, h//2, q)) to SBUF."""
            strip = strip_of(t)
            drel = t * QB - strip
            bidx = {0: 0, 128: 1, 256: 2}[drel]
            po = ps_o.tile([65, 4 * QB], f32, tag="po")
            for h in range(HPC):
                ps = ps_s.tile([128, 1024], f32, tag="ps_s")
                for c in range(4):
                    nc.tensor.matmul(
                        ps[:, c * 256:(c + 1) * 256],
                        k_pack[:, h * N + strip + c * 128: h * N + strip + (c + 1) * 128],
                        q_pack[:, h * N + t * QB: h * N + (t + 1) * QB],
                        start=True, stop=True)
                ex0 = exp_pool.tile([128, 1024], bf16, tag="ex0")
                nc.scalar.activation(ex0[:], ps[:], AF.Exp, scale=0.125)
                ex = exp_pool.tile([128, 1024], bf16, tag="ex")
                eng = nc.vector if h < 3 else nc.gpsimd
                eng.tensor_tensor(ex[:], ex0[:],
                                  band_t[:, bidx * 1024:(bidx + 1) * 1024],
                                  OP.mult)
                if debug and t == 4 and h < 2:
                    nc.vector.tensor_copy(ex_dbg[:, h * 1024:(h + 1) * 1024], ex[:])
                for c in range(4):
                    ktile = (strip + c * 128) // 128
                    nc.tensor.matmul(
                        po[:, h * QB:(h + 1) * QB],
                        v_sb[:, ktile * 260 + h * 65: ktile * 260 + h * 65 + 65],
                        ex[:, c * 256:(c + 1) * 256],
                        start=(c == 0), stop=(c == 3))
            # early drain frees the PSUM tile; free layout becomes
            # (b=h%2, a=h//2, q) so the normalize TTs pair heads (b, b+2).
            po_s = po_pool.tile([65, 4 * QB], bf16, tag="po_s")
            dst = po_s[:, :].rearrange("p (b a q) -> p a b q", b=2, a=2)
            src = po[:, :].rearrange("p (a b q) -> p a b q", a=2, b=2)
            if t % 2 == 0:
                nc.scalar.copy(dst, src)
            else:
                nc.vector.tensor_copy(dst, src)
            return t, po_s

        def emit_tail(state):
            """normalize: reciprocal of sums (DMA lane-spread) and scale."""
            t, po_s = state
            s32 = r_pool.tile([64, 16], bf16, tag="s32")
            nc.sync.dma_start(s32[:], po_s[64:65, :])
            r32 = r_pool.tile([64, 16], f32, tag="r32")
            nc.vector.reciprocal(r32[:], s32[:])
            r_row = r_pool.tile([1, 4 * QB], bf16, tag="r_row")
            nc.gpsimd.dma_start(r_row[:], r32[:])  # gpsimd DMA casts f32->bf16
            rb = r_pool.tile([64, 4 * QB], bf16, tag="rb")
            nc.gpsimd.partition_broadcast(rb[:], r_row[0:1, :])
            if debug:
                nc.vector.tensor_copy(sums_dbg[0:1, t * 1024:(t + 1) * 1024],
                                      po_s[64:65, :])
                nc.vector.tensor_copy(r_dbg[0:1, t * 1024:(t + 1) * 1024],
                                      r_row[:])
            for par in range(2):
                # partition half par holds heads {par, par+2} as (a, q) blocks
                nc.vector.tensor_tensor(
                    aoT[64 * par: 64 * par + 64, :].rearrange(
                        "p (i q) -> p i q", i=2)[:, :, t * QB:(t + 1) * QB],
                    po_s[0:64, par * 512:(par + 1) * 512].rearrange(
                        "p (a q) -> p a q", a=2),
                    rb[:, par * 512:(par + 1) * 512].rearrange(
                        "p (a q) -> p a q", a=2),
                    OP.mult)

        # tails are emitted one block behind the compute so the long
        # reciprocal chain never stalls the per-engine queues.
        pending = None
        for t in range(N // QB):
            state = emit_compute(t)
            if pending is not None:
                emit_tail(pending)
            pending = state
        emit_tail(pending)
        if debug:
            # dump intermediates to out_d and skip phase D
            nc.sync.dma_start(out_d[0:64, :], q_pack[:, 0:N])
            nc.sync.dma_start(out_d[64:128, :], k_pack[:, 0:N])
            nc.sync.dma_start(out_d[128:256, :], v_sb[:, 0:2048])
            nc.sync.dma_start(out_d[256:320, :], q_pack[:, N:2 * N])
            nc.sync.dma_start(out_d[320:384, :], k_pack[:, N:2 * N])
            nc.sync.dma_start(out_d[384:512, :], aoT[:, 0:N])
            nc.sync.dma_start(out_d[512:640, :], aoT[:, N:2 * N])
            sums_bf = dbg_pool.tile([1, 8 * 1024], bf16, name="sums_bf")
            r_bf = dbg_pool.tile([1, 8 * 1024], bf16, name="r_bf")
            nc.vector.tensor_copy(sums_bf[:], sums_dbg[:])
            nc.vector.tensor_copy(r_bf[:], r_dbg[:])
            for rr in range(4):
                nc.sync.dma_start(out_d[640 + rr:641 + rr, :],
                                  sums_bf[0:1, rr * 2048:(rr + 1) * 2048])
                nc.sync.dma_start(out_d[644 + rr:645 + rr, :],
                                  r_bf[0:1, rr * 2048:(rr + 1) * 2048])
            nc.sync.dma_start(out_d[648:776, :], ex_dbg[:])
        pc.close()

        # ------------------------------------------------ phase D: output projection
        pd = ExitStack()
        pd.enter_context(nc.named_scope("phD"))
        ps_w = pd.enter_context(tc.tile_pool(name="ps_w", bufs=4, space="PSUM"))
        out_pool = pd.enter_context(tc.tile_pool(name="outp", bufs=3))
        for m in (range(0) if debug else range(8)):
            for half in range(2):
                ob = out_pool.tile([128, 1024], bf16, tag="ob")
                for sub in range(2):
                    ch = half * 2 + sub
                    pw = ps_w.tile([128, 512], f32, tag="pw")
                    for icb in range(2):
                        nc.tensor.matmul(
                            pw[:],
                            wo_t[:, icb * D + m * 128: icb * D + (m + 1) * 128],
                            aoT[:, icb * N + ch * 512: icb * N + (ch + 1) * 512],
                            start=(icb == 0), stop=(icb == 1))
                    if sub == 0:
                        nc.scalar.copy(ob[:, sub * 512:(sub + 1) * 512], pw[:])
                    else:
                        nc.vector.tensor_copy(ob[:, sub * 512:(sub + 1) * 512], pw[:])
                nc.gpsimd.dma_start(
                    out_d[m * 128:(m + 1) * 128, half * 1024:(half + 1) * 1024], ob[:])
        pd.close()
        top.close()

    nc.compile()
    return nc


# ----------------------------------------------------------------------------
# host side
# ----------------------------------------------------------------------------
def _host_prep(x, freqs, Wq, bq, Wk, bk, Wv, bv, Wo, half):
    """Build the 8 per-core input maps (bf16 device payloads)."""
    import ml_dtypes
    bf = ml_dtypes.bfloat16

    perm = np.concatenate([np.arange(0, 64, 2), np.arange(1, 64, 2)])
    cos_f = np.cos(freqs.astype(np.float64)).astype(np.float32)
    sin_f = np.sin(freqs.astype(np.float64)).astype(np.float32)
    cosT0 = np.ascontiguousarray(cos_f[:, perm].T)
    sinT0 = np.ascontiguousarray(sin_f[:, perm].T)
    sinT0[0:32] *= -1.0
    cos_id = np.ones((64, N), np.float32)
    sin_id = np.zeros((64, N), np.float32)

    # band patterns for the three strip offsets
    p = np.arange(128)
    q = np.arange(256)
    band = np.empty((128, 3 * 1024), np.float32)
    for bi, d in enumerate((0, 128, 256)):
        for c in range(4):
            k = c * 128 + p
            keep = np.abs(k[:, None] - d - q[None, :]) <= half
            band[:, bi * 1024 + c * 256: bi * 1024 + (c + 1) * 256] = \
                np.where(keep, 1.0, 0.0)

    bv_any = bool(np.any(bv))
    bqk_any = bool(np.any(bq) or np.any(bk))

    def swiz_w(w):  # [D, SLICE] -> [128, KB*SLICE], block kb at free kb*SLICE
        return np.ascontiguousarray(
            w.reshape(KB, 128, SLICE).transpose(1, 0, 2).reshape(128, KB * SLICE))

    maps = []
    xT_pre = {}
    for b in range(B):
        # [128, (ch, kb, s)]: xT_pre[p, ch*4096+kb*512+s] = x[b, ch*512+s, kb*128+p]
        xT_pre[b] = np.ascontiguousarray(
            x[b].reshape(NCH, 512, KB, 128).transpose(3, 0, 2, 1)
            .reshape(128, NCH * KB * 512)).astype(bf)
    for core in range(8):
        b, g = core // 4, core % 4
        sl = slice(g * SLICE, (g + 1) * SLICE)
        wq_s = np.ascontiguousarray(Wq[:, sl])
        wk_s = np.ascontiguousarray(Wk[:, sl])
        bq_s = bq[sl].copy()
        bk_s = bk[sl].copy()
        if g == 0:
            wq_s = wq_s.copy(); wq_s[:, 0:64] = wq_s[:, 0:64][:, perm]
            wk_s = wk_s.copy(); wk_s[:, 0:64] = wk_s[:, 0:64][:, perm]
            bq_s[0:64] = bq_s[0:64][perm]
            bk_s[0:64] = bk_s[0:64][perm]
            cosT, sinT = cosT0, sinT0
        else:
            cosT, sinT = cos_id, sin_id
        # bias layout [128, 4]: cols (bq cb0, bq cb1, bk cb0, bk cb1)
        bqk = np.stack([bq_s[0:128], bq_s[128:256], bk_s[0:128], bk_s[128:256]],
                       axis=1).astype(np.float32)
        wo_s = Wo[sl, :].reshape(2, 128, D).transpose(1, 0, 2).reshape(128, 2 * D)
        maps.append(dict(
            xT=xT_pre[b],
            wq=swiz_w(wq_s).astype(bf), wk=swiz_w(wk_s).astype(bf),
            wv=swiz_w(np.ascontiguousarray(Wv[:, sl])).astype(bf),
            wo=np.ascontiguousarray(wo_s).astype(bf),
            bqk=bqk, cosT=cosT.astype(bf), sinT=sinT.astype(bf),
            band=band.astype(bf),
            bvrow=np.concatenate([bv[sl], np.zeros(256, np.float32)])[None, :]
            .astype(np.float32),
        ))
    return maps, bv_any, bqk_any


def _numpy_fallback(x, mask, freqs, Wq, bq, Wk, bk, Wv, bv, Wo, bo, window_size):
    """Reference math in numpy (handles arbitrary mask / window)."""
    b, n, _ = x.shape
    h, hd = H, HD

    def rope(t):
        rot = freqs.shape[-1]
        tr = t[..., :rot].reshape(b, n, -1, 2)
        t1, t2 = tr[..., 0], tr[..., 1]
        rh = np.stack((-t2, t1), -1).reshape(b, n, rot)
        return np.concatenate(
            [t[..., :rot] * np.cos(freqs) + rh * np.sin(freqs), t[..., rot:]], -1)

    q = rope(x @ Wq + bq).reshape(b, n, h, hd).transpose(0, 2, 1, 3)
    k = rope(x @ Wk + bk).reshape(b, n, h, hd).transpose(0, 2, 1, 3)
    v = (x @ Wv + bv).reshape(b, n, h, hd).transpose(0, 2, 1, 3)
    i = np.arange(n)[:, None]
    j = np.arange(n)[None, :]
    half = int(window_size) // 2
    wm = (j >= i - half) & (j <= i + half)
    fm = wm[None, None] & mask[:, None, None, :]
    s = np.einsum("bhqd,bhkd->bhqk", q, k) / np.sqrt(np.float32(hd))
    s = np.where(fm, s, np.finfo(np.float32).min)
    s = s - s.max(-1, keepdims=True)
    e = np.exp(s)
    a = e / e.sum(-1, keepdims=True)
    out = np.einsum("bhqk,bhkd->bhqd", a, v).transpose(0, 2, 1, 3).reshape(b, n, h * hd)
    out = out @ Wo + bo
    return np.where(mask[..., None], out, 0.0).astype(np.float32)


def _ensure_ntff_hook():
    """The agent image's antenv lacks axon_hooks; synthesize it so
    run_bass_kernel_spmd(trace=True) can capture NTFF profiles."""
    import sys
    import types
    try:
        from antenv.axon_hooks import get_axon_ntff_profile_hook  # noqa: F401
        return
    except ImportError:
        pass
    try:
        import antenv
        from trn_agent_boot.trn_boot import _ntff_profile_via_ctypes
        hook = _ntff_profile_via_ctypes("/opt/axon/libaxon_pjrt.so")
        mod = types.ModuleType("antenv.axon_hooks")
        mod.get_axon_ntff_profile_hook = lambda: hook
        mod.set_axon_ntff_profile_hook = lambda h: None
        sys.modules["antenv.axon_hooks"] = mod
        antenv.axon_hooks = mod
    except Exception:
        pass


def kernel(x, mask, freqs, Wq, bq, Wk, bk, Wv, bv, Wo, bo, window_size):
    global _last_results
    x = np.asarray(x, np.float32)
    mask_np = np.asarray(mask)
    freqs = np.asarray(freqs, np.float32)
    Wq = np.asarray(Wq, np.float32); Wk = np.asarray(Wk, np.float32)
    Wv = np.asarray(Wv, np.float32); Wo = np.asarray(Wo, np.float32)
    bq = np.asarray(bq, np.float32); bk = np.asarray(bk, np.float32)
    bv = np.asarray(bv, np.float32); bo = np.asarray(bo, np.float32)
    ws = int(window_size)

    if (x.shape != (B, N, D) or freqs.shape != (N, HD) or ws > 256 or ws % 2
            or not mask_np.all()):
        return _numpy_fallback(x, mask_np, freqs, Wq, bq, Wk, bk, Wv, bv, Wo, bo, ws)

    from concourse.bass_utils import run_bass_kernel_spmd

    maps, bv_any, bqk_any = _host_prep(x, freqs, Wq, bq, Wk, bk, Wv, bv, Wo, ws // 2)
    dbg = bool(int(os.environ.get("KERNEL_DEBUG", "0")))
    key = ("v2", bv_any, bqk_any, dbg)
    if key not in _CACHE:
        _CACHE[key] = _build_program(bv_any, bqk_any, debug=dbg)
    nc = _CACHE[key]

    trace = bool(int(os.environ.get("KERNEL_TRACE", "0")))
    if trace:
        _ensure_ntff_hook()
    res = run_bass_kernel_spmd(nc, maps, core_ids=list(range(8)), trace=trace)
    _last_results = res

    out = np.empty((B, N, D), np.float32)
    for b in range(B):
        acc = res.results[4 * b]["out"].astype(np.float32)
        for g in range(1, 4):
            acc = acc + res.results[4 * b + g]["out"].astype(np.float32)
        out[b] = acc.T + bo[None, :]
    out *= mask_np[..., None].astype(np.float32)
    return out
